# revision 29
# baseline (speedup 1.0000x reference)
"""Multi-head attention (B=2,S=2048,E=1024,H=16,DK=DV=64) on 8 Trainium2 cores.

Sharding: core c handles batch c//4 and head-group c%4 (4 heads = 2 pairs).
v2 design (vs baseline):
  - PE kept dense/warm: warmup matmuls at t=0, projection / transpose /
    output-projection work injected as PE "filler" between attention matmuls
    so the HAM clock stays at 2.4 GHz.
  - Scores for the two heads of a pair are issued back-to-back on disjoint
    PE row groups (rows 0-63 / 64-127) for hardware concurrency.
  - One exp per (pair, q-block, t) over [128, 1024] covering both heads.
  - ctx matmuls software-pipelined one iteration behind scores/exp.
  - ones-column denominator trick; chain B's ctx PSUM is based at partition
    63 (ones column first) so every normalization op is partition-aligned.
  - Input DMAs split into 512/1024-column pieces across the 16 queues in
    need-order; y output in bf16; host sums partials + bo.
"""

import numpy as np
import ml_dtypes

import os
import concourse.bacc as bacc
import concourse.mybir as mybir
import concourse.tile as tile
from concourse import bass_utils

BF = ml_dtypes.bfloat16
dt = mybir.dt

NCORES = 8

S, E, DK = 2048, 1024, 64
EC = E // 128          # 8 contraction chunks
NT = S // 128          # 16 seq tiles
QB = 512               # q block
NQB = S // QB          # 4
NPAIR = 2              # 2 pairs of heads per core (4 heads)


def _emit(nc, tc, inp, y_d):
    BISECT = int(os.environ.get("KBISECT", "3"))
    Exp = mybir.ActivationFunctionType.Exp
    Copy = mybir.ActivationFunctionType.Copy
    f32 = dt.float32
    bf = dt.bfloat16

    persist = tc.alloc_tile_pool(name="persist", bufs=1)

    ones = persist.tile([1, QB], bf, name="ones")
    nc.gpsimd.memset(ones[:], 1.0)
    neg3 = persist.tile([128, 1], f32, name="neg3")
    nc.gpsimd.memset(neg3[:], -3.0)
    warm = persist.tile([128, QB], bf, name="warm")
    nc.gpsimd.memset(warm[:], 0.125)

    # ---- weight/bias/identity DMAs first (small, high priority) ------------
    w_sb = {}
    for nm in ("wq", "wk", "wv"):
        w_sb[nm] = [persist.tile([128, 256], bf, name=f"{nm}{c}")
                    for c in range(EC)]
        for c in range(EC):
            nc.sync.dma_start(w_sb[nm][c][:], inp[nm][c])
    b_sb = {}
    for nm in ("bq", "bk", "bv"):
        b_sb[nm] = persist.tile([1, 256], bf, name=f"{nm}s")
        nc.sync.dma_start(b_sb[nm][:], inp[nm][:])
    wo_sb = [persist.tile([128, E], bf, name=f"wo{p}") for p in range(NPAIR)]
    for p in range(NPAIR):
        nc.sync.dma_start(wo_sb[p][:], inp["wo"][p])
    ident = persist.tile([128, 128], bf, name="ident")
    nc.sync.dma_start(ident[:], inp["ident"][:])

    # ---- big persistent tensors -------------------------------------------
    qT = [persist.tile([128, S], bf, name=f"qT{p}") for p in range(NPAIR)]
    kT = [persist.tile([128, S], bf, name=f"kT{p}") for p in range(NPAIR)]
    cT = [persist.tile([128, S], bf, name=f"cT{p}") for p in range(NPAIR)]
    vTs = [persist.tile([128, S], bf, name=f"vTs{p}") for p in range(NPAIR)]
    vA = [[persist.tile([128, 130], bf, name=f"vA{p}_{t}") for t in range(NT)]
          for p in range(NPAIR)]
    for p in range(NPAIR):
        for t in range(NT):
            nc.gpsimd.memset(vA[p][t][:], 1.0)
    mt = [persist.tile([128, S], bf, name=f"mt{t}") for t in range(NT)]
    dnrow = persist.tile([1, 2 * QB], f32, name="dnrow")
    rcprow = persist.tile([1, 2 * QB], f32, name="rcprow")
    rcpbrow = persist.tile([1, 2 * QB], bf, name="rcpbrow")
    bcs = persist.tile([128, QB], f32, name="bcs")
    bcsB = persist.tile([64, QB], f32, name="bcsB")
    caf = persist.tile([128, QB], f32, name="caf")
    cbf = persist.tile([128, QB], f32, name="cbf")
    ones64 = persist.tile([1, 64], bf, name="ones64")
    nc.gpsimd.memset(ones64[:], 1.0)

    # ---- mask piece DMAs, q-block-major need order -------------------------
    # (xq piece DMAs are interleaved below, after the prologue sections that
    #  define their rotation pool.)
    # ---- prologue: warmup + k projection (both pairs), c-outer -------------
    xkpool = tc.alloc_tile_pool(name="xk", bufs=3)
    with tc.tile_pool(name="pro1", bufs=1, space="PSUM") as pro1:
        kps = [pro1.tile([128, S], f32, tag=f"kps{p}", name=f"kps{p}")
               for p in range(NPAIR)]
        # HAM warmup: garbage matmuls into kps[0]; the bias matmuls below
        # start=True-clear has_written so none of this survives.
        for i in range(6):
            nc.tensor.matmul(kps[0][:, 0:512], warm[:, 0:128], warm[:],
                             start=True, stop=True)
        for p in range(NPAIR):
            for n0 in range(0, S, 512):
                nc.tensor.matmul(
                    kps[p][:, n0:n0 + 512],
                    b_sb["bk"][0:1, p * 128:(p + 1) * 128],
                    ones[0:1, :], start=True, stop=False)
        for c in range(EC):
            xkc = xkpool.tile([128, S], bf, tag="xk", name=f"xk{c}")
            for half in range(2):
                sl = slice(half * 1024, (half + 1) * 1024)
                nc.sync.dma_start(xkc[:, sl], inp["xk"][c][:, sl])
            for p in range(NPAIR):
                for n0 in range(0, S, 512):
                    nc.tensor.matmul(
                        kps[p][:, n0:n0 + 512],
                        w_sb["wk"][c][:, p * 128:(p + 1) * 128],
                        xkc[:, n0:n0 + 512],
                        start=False, stop=(c == EC - 1))
        for p in range(NPAIR):
            for half in range(2):
                sl = slice(half * 1024, (half + 1) * 1024)
                nc.scalar.activation(kT[p][:, sl], kps[p][:, sl], Copy)
    xkpool.release()

    # ---- prologue: v projection, transposed layout (c-outer, streams xv) ---
    xvpool = tc.alloc_tile_pool(name="xv", bufs=3)
    with tc.tile_pool(name="pro2", bufs=1, space="PSUM") as pro2:
        vps = [pro2.tile([128, S], f32, tag=f"vps{p}", name=f"vps{p}")
               for p in range(NPAIR)]
        for p in range(NPAIR):
            for n0 in range(0, S, 512):
                nc.tensor.matmul(vps[p][:, n0:n0 + 512],
                                 b_sb["bv"][0:1, p * 128:(p + 1) * 128],
                                 ones[0:1, :], start=True, stop=False)
        for c in range(EC):
            xvc = xvpool.tile([128, S], bf, tag="xv", name=f"xv{c}")
            for half in range(2):
                sl = slice(half * 1024, (half + 1) * 1024)
                nc.sync.dma_start(xvc[:, sl], inp["xv"][c][:, sl])
            for p in range(NPAIR):
                for n0 in range(0, S, 512):
                    nc.tensor.matmul(vps[p][:, n0:n0 + 512],
                                     w_sb["wv"][c][:, p * 128:(p + 1) * 128],
                                     xvc[:, n0:n0 + 512],
                                     start=False, stop=(c == EC - 1))
        for p in range(NPAIR):
            for half in range(2):
                sl = slice(half * 1024, (half + 1) * 1024)
                nc.scalar.activation(vTs[p][:, sl], vps[p][:, sl], Copy)
    xvpool.release()

    # ---- xq half-chunk pieces + mask halves in need order ------------------
    xqpool = tc.alloc_tile_pool(name="xq", bufs=8)
    xq_pc = {}
    for h in range(2):
        hsl = slice(h * 1024, (h + 1) * 1024)
        for c in range(EC):
            pc = xqpool.tile([128, 1024], bf, tag="xqp", name=f"xq{c}_{h}")
            nc.sync.dma_start(pc[:], inp["xq"][c][:, hsl])
            xq_pc[(c, h)] = pc
        if h == 0:
            for t in range(NT):
                m8 = xqpool.tile([128, 1024], dt.float8e4, tag="m8",
                                 bufs=4, name=f"m8_{t}_0")
                nc.sync.dma_start(m8[:], inp["mask"][:, t, 0:1024])
                nc.gpsimd.tensor_copy(mt[t][:, 0:1024], m8[:])
    for t in range(NT):
        m8 = xqpool.tile([128, 1024], dt.float8e4, tag="m8", bufs=4,
                         name=f"m8_{t}_1")
        nc.sync.dma_start(m8[:], inp["mask"][:, t, 1024:2048])
        nc.gpsimd.tensor_copy(mt[t][:, 1024:2048], m8[:])

    # ---- attention-phase PSUM pools ---------------------------------------
    fillp = tc.alloc_tile_pool(name="fill", bufs=1, space="PSUM")
    attnp = tc.alloc_tile_pool(name="attn", bufs=1, space="PSUM")
    espool = tc.alloc_tile_pool(name="es", bufs=3)
    ysbp = tc.alloc_tile_pool(name="ysb", bufs=2)

    # --- filler units (each emits a small batch of PE work + followups) ----
    def unit_transpose(p, t, engine_act):
        def emit():
            tp = fillp.tile([128, 128], bf, tag="tp", name=f"tp{p}_{t}")
            nc.tensor.transpose(tp[:], vTs[p][:, t * 128:(t + 1) * 128],
                                ident[:])
            eng = nc.scalar if engine_act else nc.vector
            if engine_act:
                nc.scalar.activation(vA[p][t][:, 0:64], tp[:, 0:64], Copy)
                nc.scalar.activation(vA[p][t][:, 65:129], tp[:, 64:128], Copy)
            else:
                nc.vector.tensor_copy(vA[p][t][:, 0:64], tp[:, 0:64])
                nc.vector.tensor_copy(vA[p][t][:, 65:129], tp[:, 64:128])
        return emit

    def unit_qproj(p, qb, engine_act):
        def emit():
            qsl = slice(qb * QB, (qb + 1) * QB)
            qp = fillp.tile([128, QB], f32, tag="f512", name=f"qp{p}_{qb}")
            nc.tensor.matmul(qp[:], b_sb["bq"][0:1, p * 128:(p + 1) * 128],
                             ones[0:1, :], start=True, stop=False)
            for c in range(EC):
                nc.tensor.matmul(qp[:],
                                 w_sb["wq"][c][:, p * 128:(p + 1) * 128],
                                 xq_pc[(c, qb // 2)][:, (qb % 2) * QB:
                                                     (qb % 2) * QB + QB],
                                 start=False, stop=(c == EC - 1))
            if engine_act:
                nc.scalar.activation(qT[p][:, qsl], qp[:], Copy, scale=0.125)
            else:
                nc.vector.tensor_scalar_mul(qT[p][:, qsl], qp[:], 0.125)
        return emit

    def unit_yproj(s, use_act):
        def emit():
            ysb = ysbp.tile([128, E], bf, tag="ysb", name=f"ysb{s}")
            scol = slice(s * 128, (s + 1) * 128)
            for eh in range(2):
                esl = slice(eh * 512, (eh + 1) * 512)
                yp = fillp.tile([128, QB], f32, tag="f512", name=f"yp{s}_{eh}")
                for p in range(NPAIR):
                    nc.tensor.matmul(yp[:], cT[p][:, scol], wo_sb[p][:, esl],
                                     start=(p == 0), stop=(p == NPAIR - 1))
                if use_act:
                    nc.scalar.activation(ysb[:, esl], yp[:], Copy)
                else:
                    nc.vector.tensor_copy(ysb[:, esl], yp[:])
            nc.sync.dma_start(y_d[scol, 0:512], ysb[:, 0:512])
            nc.sync.dma_start(y_d[scol, 512:1024], ysb[:, 512:1024])
        return emit

    # pre-attention: q-block 0 for both pairs (ACT copies), transposes t=0..2
    for p in range(NPAIR):
        unit_qproj(p, 0, True)()
    for t in range(3):
        for p in range(NPAIR):
            unit_transpose(p, t, True)()

    # filler queue: (deadline_iter, emit)
    fillers = []
    for t in range(3, NT):
        fillers.append((t - 2, unit_transpose(0, t, False)))
    for qb in range(1, NQB):
        fillers.append((16 * qb - 8, unit_qproj(0, qb, False)))
        fillers.append((16 * qb - 7, unit_qproj(1, qb, False)))
    for t in range(NT):
        fillers.append((20 + t, unit_transpose(1, t, False)))
    for qb in range(NQB):
        base = 64 + 16 * qb + 19
        for j in range(4):
            fillers.append((base + 3 * j, unit_yproj(4 * qb + j, False)))
    fillers.sort(key=lambda x: x[0])
    fillers = list(reversed(fillers))  # pop from end

    # ---- main attention loop ----------------------------------------------
    pend_ctx = None       # closure emitting previous iteration's ctx MMs
    pend_norm2 = None     # stage-2 normalization (muls)
    cur_ctx = [None, None]

    def make_ctx(p, qb, t, es):
        def emit():
            if t == 0:
                cur_ctx[0] = attnp.tile([128, QB], f32, tag="ctxA",
                                        name=f"ctxA{p}_{qb}")
                cur_ctx[1] = attnp.tile([128, QB], f32, tag="ctxB",
                                        name=f"ctxB{p}_{qb}")
            ca, cb = cur_ctx
            nc.tensor.matmul(ca[0:65, :], vA[p][t][:, 0:65], es[:, 0:512],
                             start=(t == 0), stop=(t == NT - 1),
                             tile_position=(0, 0))
            nc.tensor.matmul(cb[0:65, :], vA[p][t][:, 65:130],
                             es[:, 512:1024],
                             start=(t == 0), stop=(t == NT - 1),
                             tile_position=(0, 0))
            if t == NT - 1:
                # stage 1: evacuate ctx psum -> SBUF (frees the banks),
                # denominators -> row 0, reciprocal
                nc.vector.tensor_copy(dnrow[0:1, 0:QB], ca[64:65, :])
                nc.vector.tensor_copy(dnrow[0:1, QB:2 * QB], cb[64:65, :])
                nc.vector.tensor_copy(caf[0:64, :], ca[0:64, :])
                nc.vector.tensor_copy(cbf[0:64, :], cb[0:64, :])
                nc.vector.reciprocal_approx_fast(rcprow[:], dnrow[:])
                nc.vector.tensor_copy(rcpbrow[:], rcprow[:])
        return emit

    def make_norm2(p, qb, ca, cb):
        def emit():
            qsl = slice(qb * QB, (qb + 1) * QB)
            bc = fillp.tile([128, QB], f32, tag="f512", name=f"bc{p}_{qb}")
            nc.tensor.matmul(bc[0:64, :], ones64[:], rcpbrow[0:1, 0:QB],
                             start=True, stop=True)
            nc.tensor.matmul(bc[64:128, :], ones64[:], rcpbrow[0:1, QB:2 * QB],
                             start=True, stop=True)
            nc.vector.tensor_copy(bcs[0:64, :], bc[0:64, :])
            nc.vector.tensor_copy(bcsB[:], bc[64:128, :])
            nc.vector.tensor_mul(cT[p][0:64, qsl], caf[0:64, :],
                                 bcs[0:64, :])
            nc.vector.tensor_mul(cT[p][64:128, qsl], cbf[0:64, :],
                                 bcsB[:])
        return emit

    it = 0
    for p in range(NPAIR if BISECT >= 2 else 0):
        for qb in range(NQB):
            qsl = slice(qb * QB, (qb + 1) * QB)
            for t in range(NT):
                tcol = slice(t * 128, (t + 1) * 128)
                st = attnp.tile([128, 1024], f32, tag="st", bufs=2,
                                name=f"st{p}_{qb}_{t}")
                nc.tensor.matmul(st[:, 0:512], kT[p][0:64, tcol],
                                 qT[p][0:64, qsl], start=True, stop=True)
                nc.tensor.matmul(st[:, 512:1024], kT[p][64:128, tcol],
                                 qT[p][64:128, qsl], start=True, stop=True)
                # one filler unit per iteration when due
                if fillers and fillers[-1][0] <= it:
                    fillers.pop()[1]()
                # flush previous iteration's ctx (software pipeline); at block
                # boundaries this is the previous block's t=15 ctx + stage-1
                # norm, and cur_ctx still points at that block's tiles.
                if pend_ctx is not None:
                    pend_ctx()
                    pend_ctx = None
                if pend_norm2 is not None:
                    pn_p, pn_qb, pn_ca, pn_cb = pend_norm2
                    make_norm2(pn_p, pn_qb, pn_ca, pn_cb)()
                    pend_norm2 = None
                es = espool.tile([128, 1024], bf, tag="es",
                                 name=f"es{p}_{qb}_{t}")
                nc.scalar.activation(es[:], st[:], Exp, bias=neg3[:])
                nc.vector.tensor_mul(es[:, 0:512], es[:, 0:512],
                                     mt[t][:, qsl])
                nc.vector.tensor_mul(es[:, 512:1024], es[:, 512:1024],
                                     mt[t][:, qsl])
                pend_ctx = make_ctx(p, qb, t, es)
                it += 1
            # queue stage-2 normalization; fires at t==2 of the next block
            # (after the pending t=15 ctx flush), or in the tail.
            pend_norm2 = (p, qb, cur_ctx[0], cur_ctx[1])
    # ---- tail --------------------------------------------------------------
    if pend_ctx is not None:
        pend_ctx()
        pend_ctx = None
    if pend_norm2 is not None:
        pn_p, pn_qb, pn_ca, pn_cb = pend_norm2
        make_norm2(pn_p, pn_qb, pn_ca, pn_cb)()
        pend_norm2 = None
    if BISECT >= 3:
        while fillers:
            fillers.pop()[1]()
    ysbp.release()
    espool.release()
    attnp.release()
    fillp.release()
    xqpool.release()
    persist.release()


def _build():
    nc = bacc.Bacc("TRN2", target_bir_lowering=False, debug=False,
                   num_devices=NCORES)
    inp = {}
    for nm in ("xq", "xk", "xv"):
        inp[nm] = nc.dram_tensor(nm, [EC, 128, S], dt.bfloat16,
                                 kind="ExternalInput").ap()
    for nm in ("wq", "wk", "wv"):
        inp[nm] = nc.dram_tensor(nm, [EC, 128, 256], dt.bfloat16,
                                 kind="ExternalInput").ap()
    for nm in ("bq", "bk", "bv"):
        inp[nm] = nc.dram_tensor(nm, [1, 256], dt.bfloat16,
                                 kind="ExternalInput").ap()
    inp["wo"] = nc.dram_tensor("wo", [NPAIR, 128, E], dt.bfloat16,
                               kind="ExternalInput").ap()
    inp["ident"] = nc.dram_tensor("ident", [128, 128], dt.bfloat16,
                                  kind="ExternalInput").ap()
    inp["mask"] = nc.dram_tensor("mask", [128, NT, S], dt.float8e4,
                                 kind="ExternalInput").ap()
    y_d = nc.dram_tensor("y", [S, E], dt.bfloat16, kind="ExternalOutput").ap()

    with tile.TileContext(nc) as tc:
        _emit(nc, tc, inp, y_d)
    nc.compile()
    return nc


_CACHE = {}
_TRACE = False
_TRACE_CORES = (0,)
_LAST_RESULT = None


def _get_nc():
    if "nc" not in _CACHE:
        _CACHE["nc"] = _build()
    return _CACHE["nc"]


_RUNNER_CACHE = {}


def _get_runner(nc):
    """Cached jitted shard_map executable (see baseline)."""
    if id(nc) in _RUNNER_CACHE:
        return _RUNNER_CACHE[id(nc)]
    import jax
    import concourse.mybir as _mybir
    from concourse import bass2jax
    from jax.sharding import Mesh, PartitionSpec
    from jax.experimental.shard_map import shard_map

    bass2jax.install_neuronx_cc_hook()
    pid_name = nc.partition_id_tensor.name if nc.partition_id_tensor else None
    in_names, out_names, out_avals, zero_shapes = [], [], [], []
    for alloc in nc.m.functions[0].allocations:
        if not isinstance(alloc, _mybir.MemoryLocationSet):
            continue
        name = alloc.memorylocations[0].name
        if alloc.kind == "ExternalInput":
            if name != pid_name:
                in_names.append(name)
        elif alloc.kind == "ExternalOutput":
            out_names.append(name)
            shape = tuple(alloc.tensor_shape)
            dtype = _mybir.dt.np(alloc.dtype)
            out_avals.append(jax.core.ShapedArray(shape, dtype))
            zero_shapes.append((shape, dtype))
    n_params = len(in_names)
    n_outs = len(out_avals)
    all_names = in_names + out_names
    if pid_name is not None:
        all_names = all_names + [pid_name]

    def _body(*args):
        operands = list(args)
        if pid_name is not None:
            operands.append(bass2jax.partition_id_tensor())
        return tuple(bass2jax._bass_exec_p.bind(
            *operands,
            out_avals=tuple(out_avals),
            in_names=tuple(all_names),
            out_names=tuple(out_names),
            lowering_input_output_aliases=(),
            sim_require_finite=True,
            sim_require_nnan=True,
            nc=nc,
        ))

    devices = jax.devices()[:NCORES]
    mesh = Mesh(np.asarray(devices), ("core",))
    donate = tuple(range(n_params, n_params + n_outs))
    sharded = jax.jit(
        shard_map(_body, mesh=mesh,
                  in_specs=(PartitionSpec("core"),) * (n_params + n_outs),
                  out_specs=(PartitionSpec("core"),) * n_outs,
                  check_rep=False),
        donate_argnums=donate, keep_unused=True)

    def run(in_maps):
        concat_in = [np.concatenate([np.asarray(m[nm]) for m in in_maps], axis=0)
                     for nm in in_names]
        concat_zeros = [np.zeros((NCORES * s[0], *s[1:]), d)
                        for s, d in zero_shapes]
        outs = sharded(*concat_in, *concat_zeros)
        return [
            {nm: np.asarray(outs[i]).reshape(NCORES, *out_avals[i].shape)[c]
             for i, nm in enumerate(out_names)}
            for c in range(NCORES)
        ]

    _RUNNER_CACHE[id(nc)] = run
    return run


def run_sharded(query, key, value, mask, Wq, bq, Wk, bk, Wv, bv, Wo, bo):
    global _LAST_RESULT
    query, key, value = (np.asarray(a, np.float32) for a in (query, key, value))
    mask = np.asarray(mask)
    Wq, bq, Wk, bk, Wv, bv, Wo, bo = (
        np.asarray(a, np.float32) for a in (Wq, bq, Wk, bk, Wv, bv, Wo, bo))

    B = query.shape[0]
    GPB = NCORES // B                 # cores per batch
    DKL = 256                         # local head dims per core

    nc = _get_nc()

    ident = np.eye(128, dtype=BF)
    xb = {}
    for b in range(B):
        xb[b] = {
            "xq": np.ascontiguousarray(query[b].T).astype(BF).reshape(EC, 128, S),
            "xk": np.ascontiguousarray(key[b].T).astype(BF).reshape(EC, 128, S),
            "xv": np.ascontiguousarray(value[b].T).astype(BF).reshape(EC, 128, S),
            "mask": np.ascontiguousarray(
                mask[b].reshape(S, NT, 128).transpose(2, 1, 0)).astype(
                    ml_dtypes.float8_e4m3fn),
        }

    in_maps = []
    for c in range(NCORES):
        b, g = c // GPB, c % GPB
        sl = slice(g * DKL, (g + 1) * DKL)
        in_maps.append({
            **xb[b],
            "wq": np.ascontiguousarray(Wq[:, sl]).astype(BF).reshape(EC, 128, DKL),
            "wk": np.ascontiguousarray(Wk[:, sl]).astype(BF).reshape(EC, 128, DKL),
            "wv": np.ascontiguousarray(Wv[:, sl]).astype(BF).reshape(EC, 128, DKL),
            "bq": bq[sl].astype(BF).reshape(1, DKL),
            "bk": bk[sl].astype(BF).reshape(1, DKL),
            "bv": bv[sl].astype(BF).reshape(1, DKL),
            "wo": np.ascontiguousarray(Wo[sl, :]).astype(BF).reshape(
                NPAIR, 128, E),
            "ident": ident,
        })

    if _TRACE:
        res = bass_utils.run_bass_kernel_spmd(
            nc, in_maps, core_ids=list(range(NCORES)),
            trace=True, trace_cores=list(_TRACE_CORES))
        _LAST_RESULT = res
        results = res.results
    else:
        results = _get_runner(nc)(in_maps)

    y = np.zeros((B, S, E), np.float32)
    for c in range(NCORES):
        y[c // GPB] += results[c]["y"].astype(np.float32)
    y += bo.astype(np.float32)
    return y


def kernel(**inputs):
    return run_sharded(
        inputs["query"], inputs["key"], inputs["value"], inputs["mask"],
        inputs["Wq"], inputs["bq"], inputs["Wk"], inputs["bk"],
        inputs["Wv"], inputs["bv"], inputs["Wo"], inputs["bo"])


# revision 30
# speedup vs baseline: 1.2767x; 1.2767x over previous
"""Multi-head attention (B=2,S=2048,E=1024,H=16,DK=DV=64) on 8 Trainium2 cores.

Sharding: core c handles batch c//4 and head-group c%4 (4 heads = 2 pairs).
v2 design (vs baseline):
  - PE kept dense/warm: warmup matmuls at t=0, projection / transpose /
    output-projection work injected as PE "filler" between attention matmuls
    so the HAM clock stays at 2.4 GHz.
  - Scores for the two heads of a pair are issued back-to-back on disjoint
    PE row groups (rows 0-63 / 64-127) for hardware concurrency.
  - One exp per (pair, q-block, t) over [128, 1024] covering both heads.
  - ctx matmuls software-pipelined one iteration behind scores/exp.
  - ones-column denominator trick; chain B's ctx PSUM is based at partition
    63 (ones column first) so every normalization op is partition-aligned.
  - Input DMAs split into 512/1024-column pieces across the 16 queues in
    need-order; y output in bf16; host sums partials + bo.
"""

import numpy as np
import ml_dtypes

import os
import concourse.bacc as bacc
import concourse.mybir as mybir
import concourse.tile as tile
from concourse import bass_utils

BF = ml_dtypes.bfloat16
dt = mybir.dt

NCORES = 8

S, E, DK = 2048, 1024, 64
EC = E // 128          # 8 contraction chunks
NT = S // 128          # 16 seq tiles
QB = 512               # q block
NQB = S // QB          # 4
NPAIR = 2              # 2 pairs of heads per core (4 heads)


def _emit(nc, tc, inp, y_d):
    BISECT = int(os.environ.get("KBISECT", "3"))
    Exp = mybir.ActivationFunctionType.Exp
    Copy = mybir.ActivationFunctionType.Copy
    f32 = dt.float32
    bf = dt.bfloat16

    persist = tc.alloc_tile_pool(name="persist", bufs=1)

    ones = persist.tile([1, QB], bf, name="ones")
    nc.gpsimd.memset(ones[:], 1.0)
    neg3 = persist.tile([128, 1], f32, name="neg3")
    nc.gpsimd.memset(neg3[:], -3.0)
    warm = persist.tile([128, QB], bf, name="warm")
    nc.gpsimd.memset(warm[:], 0.125)

    # ---- weight/bias/identity DMAs first (small, high priority) ------------
    w_sb = {}
    for nm in ("wq", "wk", "wv"):
        w_sb[nm] = [persist.tile([128, 256], bf, name=f"{nm}{c}")
                    for c in range(EC)]
        for c in range(EC):
            nc.sync.dma_start(w_sb[nm][c][:], inp[nm][c])
    b_sb = {}
    for nm in ("bq", "bk", "bv"):
        b_sb[nm] = persist.tile([1, 256], bf, name=f"{nm}s")
        nc.sync.dma_start(b_sb[nm][:], inp[nm][:])
    wo_sb = [persist.tile([128, E], bf, name=f"wo{p}") for p in range(NPAIR)]
    for p in range(NPAIR):
        nc.sync.dma_start(wo_sb[p][:], inp["wo"][p])
    ident = persist.tile([128, 128], bf, name="ident")
    nc.sync.dma_start(ident[:], inp["ident"][:])

    # ---- big persistent tensors -------------------------------------------
    qT = [persist.tile([128, S], bf, name=f"qT{p}") for p in range(NPAIR)]
    kT = [persist.tile([128, S], bf, name=f"kT{p}") for p in range(NPAIR)]
    cT = [persist.tile([128, S], bf, name=f"cT{p}") for p in range(NPAIR)]
    vTs = [persist.tile([128, S], bf, name=f"vTs{p}") for p in range(NPAIR)]
    vA = [[persist.tile([128, 130], bf, name=f"vA{p}_{t}") for t in range(NT)]
          for p in range(NPAIR)]
    for p in range(NPAIR):
        for t in range(NT):
            nc.gpsimd.memset(vA[p][t][:], 1.0)
    mt = [persist.tile([128, S], bf, name=f"mt{t}") for t in range(NT)]
    dnrow = persist.tile([1, 2 * QB], f32, name="dnrow")
    rcprow = persist.tile([1, 2 * QB], f32, name="rcprow")
    rcpbrow = persist.tile([1, 2 * QB], bf, name="rcpbrow")
    bcs = persist.tile([128, QB], f32, name="bcs")
    bcsB = persist.tile([64, QB], f32, name="bcsB")
    caf = persist.tile([128, QB], f32, name="caf")
    cbf = persist.tile([128, QB], f32, name="cbf")
    ones64 = persist.tile([1, 64], bf, name="ones64")
    nc.gpsimd.memset(ones64[:], 1.0)

    # ---- mask piece DMAs, q-block-major need order -------------------------
    # (xq piece DMAs are interleaved below, after the prologue sections that
    #  define their rotation pool.)
    # ---- prologue: warmup + k projection (both pairs), c-outer -------------
    xkpool = tc.alloc_tile_pool(name="xk", bufs=3)
    with tc.tile_pool(name="pro1", bufs=1, space="PSUM") as pro1:
        kps = [pro1.tile([128, S], f32, tag=f"kps{p}", name=f"kps{p}")
               for p in range(NPAIR)]
        # HAM warmup: garbage matmuls into kps[0]; the bias matmuls below
        # start=True-clear has_written so none of this survives.
        for i in range(6):
            nc.tensor.matmul(kps[0][:, 0:512], warm[:, 0:128], warm[:],
                             start=True, stop=True)
        for p in range(NPAIR):
            for n0 in range(0, S, 512):
                nc.tensor.matmul(
                    kps[p][:, n0:n0 + 512],
                    b_sb["bk"][0:1, p * 128:(p + 1) * 128],
                    ones[0:1, :], start=True, stop=False)
        for c in range(EC):
            xkc = xkpool.tile([128, S], bf, tag="xk", name=f"xk{c}")
            for half in range(2):
                sl = slice(half * 1024, (half + 1) * 1024)
                nc.sync.dma_start(xkc[:, sl], inp["xk"][c][:, sl])
            for p in range(NPAIR):
                for n0 in range(0, S, 512):
                    nc.tensor.matmul(
                        kps[p][:, n0:n0 + 512],
                        w_sb["wk"][c][:, p * 128:(p + 1) * 128],
                        xkc[:, n0:n0 + 512],
                        start=False, stop=(c == EC - 1))
        for p in range(NPAIR):
            for half in range(2):
                sl = slice(half * 1024, (half + 1) * 1024)
                nc.scalar.activation(kT[p][:, sl], kps[p][:, sl], Copy)
    xkpool.release()

    # ---- prologue: v projection, transposed layout (c-outer, streams xv) ---
    xvpool = tc.alloc_tile_pool(name="xv", bufs=3)
    with tc.tile_pool(name="pro2", bufs=1, space="PSUM") as pro2:
        vps = [pro2.tile([128, S], f32, tag=f"vps{p}", name=f"vps{p}")
               for p in range(NPAIR)]
        for p in range(NPAIR):
            for n0 in range(0, S, 512):
                nc.tensor.matmul(vps[p][:, n0:n0 + 512],
                                 b_sb["bv"][0:1, p * 128:(p + 1) * 128],
                                 ones[0:1, :], start=True, stop=False)
        for c in range(EC):
            xvc = xvpool.tile([128, S], bf, tag="xv", name=f"xv{c}")
            for half in range(2):
                sl = slice(half * 1024, (half + 1) * 1024)
                nc.sync.dma_start(xvc[:, sl], inp["xv"][c][:, sl])
            for p in range(NPAIR):
                for n0 in range(0, S, 512):
                    nc.tensor.matmul(vps[p][:, n0:n0 + 512],
                                     w_sb["wv"][c][:, p * 128:(p + 1) * 128],
                                     xvc[:, n0:n0 + 512],
                                     start=False, stop=(c == EC - 1))
        for p in range(NPAIR):
            for half in range(2):
                sl = slice(half * 1024, (half + 1) * 1024)
                nc.scalar.activation(vTs[p][:, sl], vps[p][:, sl], Copy)
    xvpool.release()

    # ---- xq half-chunk pieces + mask halves in need order ------------------
    xqpool = tc.alloc_tile_pool(name="xq", bufs=8)
    xq_pc = {}
    for h in range(2):
        hsl = slice(h * 1024, (h + 1) * 1024)
        for c in range(EC):
            pc = xqpool.tile([128, 1024], bf, tag="xqp", name=f"xq{c}_{h}")
            nc.sync.dma_start(pc[:], inp["xq"][c][:, hsl])
            xq_pc[(c, h)] = pc
        if h == 0:
            for t in range(NT):
                nc.sync.dma_start(mt[t][:, 0:1024], inp["mask"][:, t, 0:1024])
    for t in range(NT):
        nc.sync.dma_start(mt[t][:, 1024:2048], inp["mask"][:, t, 1024:2048])

    # ---- attention-phase PSUM pools ---------------------------------------
    fillp = tc.alloc_tile_pool(name="fill", bufs=1, space="PSUM")
    attnp = tc.alloc_tile_pool(name="attn", bufs=1, space="PSUM")
    espool = tc.alloc_tile_pool(name="es", bufs=3)
    ysbp = tc.alloc_tile_pool(name="ysb", bufs=2)

    # --- filler units (each emits a small batch of PE work + followups) ----
    def unit_transpose(p, t, engine_act):
        def emit():
            tp = fillp.tile([128, 128], bf, tag="tp", name=f"tp{p}_{t}")
            nc.tensor.transpose(tp[:], vTs[p][:, t * 128:(t + 1) * 128],
                                ident[:])
            eng = nc.scalar if engine_act else nc.vector
            if engine_act:
                nc.scalar.activation(vA[p][t][:, 0:64], tp[:, 0:64], Copy)
                nc.scalar.activation(vA[p][t][:, 65:129], tp[:, 64:128], Copy)
            else:
                nc.vector.tensor_copy(vA[p][t][:, 0:64], tp[:, 0:64])
                nc.vector.tensor_copy(vA[p][t][:, 65:129], tp[:, 64:128])
        return emit

    def unit_qproj(p, qb, engine_act):
        def emit():
            qsl = slice(qb * QB, (qb + 1) * QB)
            qp = fillp.tile([128, QB], f32, tag="f512", name=f"qp{p}_{qb}")
            nc.tensor.matmul(qp[:], b_sb["bq"][0:1, p * 128:(p + 1) * 128],
                             ones[0:1, :], start=True, stop=False)
            for c in range(EC):
                nc.tensor.matmul(qp[:],
                                 w_sb["wq"][c][:, p * 128:(p + 1) * 128],
                                 xq_pc[(c, qb // 2)][:, (qb % 2) * QB:
                                                     (qb % 2) * QB + QB],
                                 start=False, stop=(c == EC - 1))
            if engine_act:
                nc.scalar.activation(qT[p][:, qsl], qp[:], Copy, scale=0.125)
            else:
                nc.vector.tensor_scalar_mul(qT[p][:, qsl], qp[:], 0.125)
        return emit

    def unit_yproj(s, use_act):
        def emit():
            ysb = ysbp.tile([128, E], bf, tag="ysb", name=f"ysb{s}")
            scol = slice(s * 128, (s + 1) * 128)
            for eh in range(2):
                esl = slice(eh * 512, (eh + 1) * 512)
                yp = fillp.tile([128, QB], f32, tag="f512", name=f"yp{s}_{eh}")
                for p in range(NPAIR):
                    nc.tensor.matmul(yp[:], cT[p][:, scol], wo_sb[p][:, esl],
                                     start=(p == 0), stop=(p == NPAIR - 1))
                if use_act:
                    nc.scalar.activation(ysb[:, esl], yp[:], Copy)
                else:
                    nc.vector.tensor_copy(ysb[:, esl], yp[:])
            nc.sync.dma_start(y_d[scol, 0:512], ysb[:, 0:512])
            nc.sync.dma_start(y_d[scol, 512:1024], ysb[:, 512:1024])
        return emit

    # pre-attention: q-block 0 for both pairs (ACT copies), transposes t=0..2
    for p in range(NPAIR):
        unit_qproj(p, 0, True)()
    for t in range(3):
        for p in range(NPAIR):
            unit_transpose(p, t, True)()

    # filler queue: (deadline_iter, emit)
    fillers = []
    for t in range(3, NT):
        fillers.append((t - 2, unit_transpose(0, t, False)))
    for qb in range(1, NQB):
        fillers.append((16 * qb - 8, unit_qproj(0, qb, False)))
        fillers.append((16 * qb - 7, unit_qproj(1, qb, False)))
    for t in range(NT):
        fillers.append((20 + t, unit_transpose(1, t, False)))
    for qb in range(NQB):
        base = 64 + 16 * qb + 19
        for j in range(4):
            fillers.append((base + 3 * j, unit_yproj(4 * qb + j, False)))
    fillers.sort(key=lambda x: x[0])
    fillers = list(reversed(fillers))  # pop from end

    # ---- main attention loop ----------------------------------------------
    pend_ctx = None       # closure emitting previous iteration's ctx MMs
    pend_norm2 = None     # stage-2 normalization (muls)
    cur_ctx = [None, None]

    def make_ctx(p, qb, t, es):
        def emit():
            if t == 0:
                cur_ctx[0] = attnp.tile([128, QB], f32, tag="ctxA",
                                        name=f"ctxA{p}_{qb}")
                cur_ctx[1] = attnp.tile([128, QB], f32, tag="ctxB",
                                        name=f"ctxB{p}_{qb}")
            ca, cb = cur_ctx
            nc.tensor.matmul(ca[0:65, :], vA[p][t][:, 0:65], es[:, 0:512],
                             start=(t == 0), stop=(t == NT - 1),
                             tile_position=(0, 0))
            nc.tensor.matmul(cb[0:65, :], vA[p][t][:, 65:130],
                             es[:, 512:1024],
                             start=(t == 0), stop=(t == NT - 1),
                             tile_position=(0, 0))
            if t == NT - 1:
                # stage 1: evacuate ctx psum -> SBUF (frees the banks),
                # denominators -> row 0, reciprocal
                nc.vector.tensor_copy(dnrow[0:1, 0:QB], ca[64:65, :])
                nc.vector.tensor_copy(dnrow[0:1, QB:2 * QB], cb[64:65, :])
                nc.vector.tensor_copy(caf[0:64, :], ca[0:64, :])
                nc.vector.tensor_copy(cbf[0:64, :], cb[0:64, :])
                nc.vector.reciprocal_approx_fast(rcprow[:], dnrow[:])
                nc.vector.tensor_copy(rcpbrow[:], rcprow[:])
        return emit

    def make_norm2(p, qb, ca, cb):
        def emit():
            qsl = slice(qb * QB, (qb + 1) * QB)
            bc = fillp.tile([128, QB], f32, tag="f512", name=f"bc{p}_{qb}")
            nc.tensor.matmul(bc[0:64, :], ones64[:], rcpbrow[0:1, 0:QB],
                             start=True, stop=True)
            nc.tensor.matmul(bc[64:128, :], ones64[:], rcpbrow[0:1, QB:2 * QB],
                             start=True, stop=True)
            nc.vector.tensor_copy(bcs[0:64, :], bc[0:64, :])
            nc.vector.tensor_copy(bcsB[:], bc[64:128, :])
            nc.vector.tensor_mul(cT[p][0:64, qsl], caf[0:64, :],
                                 bcs[0:64, :])
            nc.vector.tensor_mul(cT[p][64:128, qsl], cbf[0:64, :],
                                 bcsB[:])
        return emit

    it = 0
    for p in range(NPAIR if BISECT >= 2 else 0):
        for qb in range(NQB):
            qsl = slice(qb * QB, (qb + 1) * QB)
            for t in range(NT):
                tcol = slice(t * 128, (t + 1) * 128)
                st = attnp.tile([128, 1024], f32, tag="st", bufs=2,
                                name=f"st{p}_{qb}_{t}")
                nc.tensor.matmul(st[:, 0:512], kT[p][0:64, tcol],
                                 qT[p][0:64, qsl], start=True, stop=True)
                nc.tensor.matmul(st[:, 512:1024], kT[p][64:128, tcol],
                                 qT[p][64:128, qsl], start=True, stop=True)
                # one filler unit per iteration when due
                if fillers and fillers[-1][0] <= it:
                    fillers.pop()[1]()
                # flush previous iteration's ctx (software pipeline); at block
                # boundaries this is the previous block's t=15 ctx + stage-1
                # norm, and cur_ctx still points at that block's tiles.
                if pend_ctx is not None:
                    pend_ctx()
                    pend_ctx = None
                if pend_norm2 is not None:
                    pn_p, pn_qb, pn_ca, pn_cb = pend_norm2
                    make_norm2(pn_p, pn_qb, pn_ca, pn_cb)()
                    pend_norm2 = None
                es = espool.tile([128, 1024], bf, tag="es",
                                 name=f"es{p}_{qb}_{t}")
                nc.scalar.activation(es[:], st[:], Exp, bias=neg3[:])
                nc.vector.tensor_mul(es[:, 0:512], es[:, 0:512],
                                     mt[t][:, qsl])
                nc.vector.tensor_mul(es[:, 512:1024], es[:, 512:1024],
                                     mt[t][:, qsl])
                pend_ctx = make_ctx(p, qb, t, es)
                it += 1
            # queue stage-2 normalization; fires at t==2 of the next block
            # (after the pending t=15 ctx flush), or in the tail.
            pend_norm2 = (p, qb, cur_ctx[0], cur_ctx[1])
    # ---- tail --------------------------------------------------------------
    if pend_ctx is not None:
        pend_ctx()
        pend_ctx = None
    if pend_norm2 is not None:
        pn_p, pn_qb, pn_ca, pn_cb = pend_norm2
        make_norm2(pn_p, pn_qb, pn_ca, pn_cb)()
        pend_norm2 = None
    if BISECT >= 3:
        while fillers:
            fillers.pop()[1]()
    ysbp.release()
    espool.release()
    attnp.release()
    fillp.release()
    xqpool.release()
    persist.release()


def _build():
    nc = bacc.Bacc("TRN2", target_bir_lowering=False, debug=False,
                   num_devices=NCORES)
    inp = {}
    for nm in ("xq", "xk", "xv"):
        inp[nm] = nc.dram_tensor(nm, [EC, 128, S], dt.bfloat16,
                                 kind="ExternalInput").ap()
    for nm in ("wq", "wk", "wv"):
        inp[nm] = nc.dram_tensor(nm, [EC, 128, 256], dt.bfloat16,
                                 kind="ExternalInput").ap()
    for nm in ("bq", "bk", "bv"):
        inp[nm] = nc.dram_tensor(nm, [1, 256], dt.bfloat16,
                                 kind="ExternalInput").ap()
    inp["wo"] = nc.dram_tensor("wo", [NPAIR, 128, E], dt.bfloat16,
                               kind="ExternalInput").ap()
    inp["ident"] = nc.dram_tensor("ident", [128, 128], dt.bfloat16,
                                  kind="ExternalInput").ap()
    inp["mask"] = nc.dram_tensor("mask", [128, NT, S], dt.bfloat16,
                                 kind="ExternalInput").ap()
    y_d = nc.dram_tensor("y", [S, E], dt.bfloat16, kind="ExternalOutput").ap()

    with tile.TileContext(nc) as tc:
        _emit(nc, tc, inp, y_d)
    nc.compile()
    return nc


_CACHE = {}
_TRACE = False
_TRACE_CORES = (0,)
_LAST_RESULT = None


def _get_nc():
    if "nc" not in _CACHE:
        _CACHE["nc"] = _build()
    return _CACHE["nc"]


_RUNNER_CACHE = {}


def _get_runner(nc):
    """Cached jitted shard_map executable (see baseline)."""
    if id(nc) in _RUNNER_CACHE:
        return _RUNNER_CACHE[id(nc)]
    import jax
    import concourse.mybir as _mybir
    from concourse import bass2jax
    from jax.sharding import Mesh, PartitionSpec
    from jax.experimental.shard_map import shard_map

    bass2jax.install_neuronx_cc_hook()
    pid_name = nc.partition_id_tensor.name if nc.partition_id_tensor else None
    in_names, out_names, out_avals, zero_shapes = [], [], [], []
    for alloc in nc.m.functions[0].allocations:
        if not isinstance(alloc, _mybir.MemoryLocationSet):
            continue
        name = alloc.memorylocations[0].name
        if alloc.kind == "ExternalInput":
            if name != pid_name:
                in_names.append(name)
        elif alloc.kind == "ExternalOutput":
            out_names.append(name)
            shape = tuple(alloc.tensor_shape)
            dtype = _mybir.dt.np(alloc.dtype)
            out_avals.append(jax.core.ShapedArray(shape, dtype))
            zero_shapes.append((shape, dtype))
    n_params = len(in_names)
    n_outs = len(out_avals)
    all_names = in_names + out_names
    if pid_name is not None:
        all_names = all_names + [pid_name]

    def _body(*args):
        operands = list(args)
        if pid_name is not None:
            operands.append(bass2jax.partition_id_tensor())
        return tuple(bass2jax._bass_exec_p.bind(
            *operands,
            out_avals=tuple(out_avals),
            in_names=tuple(all_names),
            out_names=tuple(out_names),
            lowering_input_output_aliases=(),
            sim_require_finite=True,
            sim_require_nnan=True,
            nc=nc,
        ))

    devices = jax.devices()[:NCORES]
    mesh = Mesh(np.asarray(devices), ("core",))
    donate = tuple(range(n_params, n_params + n_outs))
    sharded = jax.jit(
        shard_map(_body, mesh=mesh,
                  in_specs=(PartitionSpec("core"),) * (n_params + n_outs),
                  out_specs=(PartitionSpec("core"),) * n_outs,
                  check_rep=False),
        donate_argnums=donate, keep_unused=True)

    def run(in_maps):
        concat_in = [np.concatenate([np.asarray(m[nm]) for m in in_maps], axis=0)
                     for nm in in_names]
        concat_zeros = [np.zeros((NCORES * s[0], *s[1:]), d)
                        for s, d in zero_shapes]
        outs = sharded(*concat_in, *concat_zeros)
        return [
            {nm: np.asarray(outs[i]).reshape(NCORES, *out_avals[i].shape)[c]
             for i, nm in enumerate(out_names)}
            for c in range(NCORES)
        ]

    _RUNNER_CACHE[id(nc)] = run
    return run


def run_sharded(query, key, value, mask, Wq, bq, Wk, bk, Wv, bv, Wo, bo):
    global _LAST_RESULT
    query, key, value = (np.asarray(a, np.float32) for a in (query, key, value))
    mask = np.asarray(mask)
    Wq, bq, Wk, bk, Wv, bv, Wo, bo = (
        np.asarray(a, np.float32) for a in (Wq, bq, Wk, bk, Wv, bv, Wo, bo))

    B = query.shape[0]
    GPB = NCORES // B                 # cores per batch
    DKL = 256                         # local head dims per core

    nc = _get_nc()

    ident = np.eye(128, dtype=BF)
    xb = {}
    for b in range(B):
        xb[b] = {
            "xq": np.ascontiguousarray(query[b].T).astype(BF).reshape(EC, 128, S),
            "xk": np.ascontiguousarray(key[b].T).astype(BF).reshape(EC, 128, S),
            "xv": np.ascontiguousarray(value[b].T).astype(BF).reshape(EC, 128, S),
            "mask": np.ascontiguousarray(
                mask[b].reshape(S, NT, 128).transpose(2, 1, 0)).astype(BF),
        }

    in_maps = []
    for c in range(NCORES):
        b, g = c // GPB, c % GPB
        sl = slice(g * DKL, (g + 1) * DKL)
        in_maps.append({
            **xb[b],
            "wq": np.ascontiguousarray(Wq[:, sl]).astype(BF).reshape(EC, 128, DKL),
            "wk": np.ascontiguousarray(Wk[:, sl]).astype(BF).reshape(EC, 128, DKL),
            "wv": np.ascontiguousarray(Wv[:, sl]).astype(BF).reshape(EC, 128, DKL),
            "bq": bq[sl].astype(BF).reshape(1, DKL),
            "bk": bk[sl].astype(BF).reshape(1, DKL),
            "bv": bv[sl].astype(BF).reshape(1, DKL),
            "wo": np.ascontiguousarray(Wo[sl, :]).astype(BF).reshape(
                NPAIR, 128, E),
            "ident": ident,
        })

    if _TRACE:
        res = bass_utils.run_bass_kernel_spmd(
            nc, in_maps, core_ids=list(range(NCORES)),
            trace=True, trace_cores=list(_TRACE_CORES))
        _LAST_RESULT = res
        results = res.results
    else:
        results = _get_runner(nc)(in_maps)

    y = np.zeros((B, S, E), np.float32)
    for c in range(NCORES):
        y[c // GPB] += results[c]["y"].astype(np.float32)
    y += bo.astype(np.float32)
    return y


def kernel(**inputs):
    return run_sharded(
        inputs["query"], inputs["key"], inputs["value"], inputs["mask"],
        inputs["Wq"], inputs["bq"], inputs["Wk"], inputs["bk"],
        inputs["Wv"], inputs["bv"], inputs["Wo"], inputs["bo"])


# revision 31
# speedup vs baseline: 1.2916x; 1.0117x over previous
"""Multi-head attention (B=2,S=2048,E=1024,H=16,DK=DV=64) on 8 Trainium2 cores.

Sharding: core c handles batch c//4 and head-group c%4 (4 heads = 2 pairs).
v2 design (vs baseline):
  - PE kept dense/warm: warmup matmuls at t=0, projection / transpose /
    output-projection work injected as PE "filler" between attention matmuls
    so the HAM clock stays at 2.4 GHz.
  - Scores for the two heads of a pair are issued back-to-back on disjoint
    PE row groups (rows 0-63 / 64-127) for hardware concurrency.
  - One exp per (pair, q-block, t) over [128, 1024] covering both heads.
  - ctx matmuls software-pipelined one iteration behind scores/exp.
  - ones-column denominator trick; chain B's ctx PSUM is based at partition
    63 (ones column first) so every normalization op is partition-aligned.
  - Input DMAs split into 512/1024-column pieces across the 16 queues in
    need-order; y output in bf16; host sums partials + bo.
"""

import numpy as np
import ml_dtypes

import os
import concourse.bacc as bacc
import concourse.mybir as mybir
import concourse.tile as tile
from concourse import bass_utils

BF = ml_dtypes.bfloat16
dt = mybir.dt

NCORES = 8

S, E, DK = 2048, 1024, 64
EC = E // 128          # 8 contraction chunks
NT = S // 128          # 16 seq tiles
QB = 512               # q block
NQB = S // QB          # 4
NPAIR = 2              # 2 pairs of heads per core (4 heads)


def _emit(nc, tc, inp, y_d):
    BISECT = int(os.environ.get("KBISECT", "3"))
    Exp = mybir.ActivationFunctionType.Exp
    Copy = mybir.ActivationFunctionType.Copy
    f32 = dt.float32
    bf = dt.bfloat16

    persist = tc.alloc_tile_pool(name="persist", bufs=1)

    ones = persist.tile([1, QB], bf, name="ones")
    nc.gpsimd.memset(ones[:], 1.0)
    neg3 = persist.tile([128, 1], f32, name="neg3")
    nc.gpsimd.memset(neg3[:], -3.0)
    warm = persist.tile([128, QB], bf, name="warm")
    nc.gpsimd.memset(warm[:], 0.125)

    # ---- weight/bias/identity DMAs first (small, high priority) ------------
    w_sb = {}
    for nm in ("wq", "wk", "wv"):
        w_sb[nm] = [persist.tile([128, 256], bf, name=f"{nm}{c}")
                    for c in range(EC)]
        for c in range(EC):
            nc.sync.dma_start(w_sb[nm][c][:], inp[nm][c])
    b_sb = {}
    for nm in ("bq", "bk", "bv"):
        b_sb[nm] = persist.tile([1, 256], bf, name=f"{nm}s")
        nc.sync.dma_start(b_sb[nm][:], inp[nm][:])
    wo_sb = [persist.tile([128, E], bf, name=f"wo{p}") for p in range(NPAIR)]
    for p in range(NPAIR):
        nc.sync.dma_start(wo_sb[p][:], inp["wo"][p])
    ident = persist.tile([128, 128], bf, name="ident")
    nc.sync.dma_start(ident[:], inp["ident"][:])

    # ---- big persistent tensors -------------------------------------------
    qT = [persist.tile([128, S], bf, name=f"qT{p}") for p in range(NPAIR)]
    kT = [persist.tile([128, S], bf, name=f"kT{p}") for p in range(NPAIR)]
    cT = [persist.tile([128, S], bf, name=f"cT{p}") for p in range(NPAIR)]
    vTs = [persist.tile([128, S], bf, name=f"vTs{p}") for p in range(NPAIR)]
    vA = [[persist.tile([128, 130], bf, name=f"vA{p}_{t}") for t in range(NT)]
          for p in range(NPAIR)]
    for p in range(NPAIR):
        for t in range(NT):
            nc.gpsimd.memset(vA[p][t][:], 1.0)
    mt = [persist.tile([128, S], bf, name=f"mt{t}") for t in range(NT)]
    dnrow = persist.tile([1, 2 * QB], f32, name="dnrow")
    rcprow = persist.tile([1, 2 * QB], f32, name="rcprow")
    rcpbrow = persist.tile([1, 2 * QB], bf, name="rcpbrow")
    bcs = persist.tile([128, QB], f32, name="bcs")
    bcsB = persist.tile([64, QB], f32, name="bcsB")
    caf = persist.tile([128, QB], f32, name="caf")
    cbf = persist.tile([128, QB], f32, name="cbf")
    ones64 = persist.tile([1, 64], bf, name="ones64")
    nc.gpsimd.memset(ones64[:], 1.0)

    # ---- mask piece DMAs, q-block-major need order -------------------------
    # (xq piece DMAs are interleaved below, after the prologue sections that
    #  define their rotation pool.)
    # ---- prologue: warmup + k projection (both pairs), c-outer -------------
    xkpool = tc.alloc_tile_pool(name="xk", bufs=3)
    with tc.tile_pool(name="pro1", bufs=1, space="PSUM") as pro1:
        kps = [pro1.tile([128, S], f32, tag=f"kps{p}", name=f"kps{p}")
               for p in range(NPAIR)]
        # HAM warmup: garbage matmuls into kps[0]; the bias matmuls below
        # start=True-clear has_written so none of this survives.
        for i in range(6):
            nc.tensor.matmul(kps[0][:, 0:512], warm[:, 0:128], warm[:],
                             start=True, stop=True)
        for p in range(NPAIR):
            for n0 in range(0, S, 512):
                nc.tensor.matmul(
                    kps[p][:, n0:n0 + 512],
                    b_sb["bk"][0:1, p * 128:(p + 1) * 128],
                    ones[0:1, :], start=True, stop=False)
        for c in range(EC):
            xkc = xkpool.tile([128, S], bf, tag="xk", name=f"xk{c}")
            for half in range(2):
                sl = slice(half * 1024, (half + 1) * 1024)
                nc.sync.dma_start(xkc[:, sl], inp["xk"][c][:, sl])
            for p in range(NPAIR):
                for n0 in range(0, S, 512):
                    nc.tensor.matmul(
                        kps[p][:, n0:n0 + 512],
                        w_sb["wk"][c][:, p * 128:(p + 1) * 128],
                        xkc[:, n0:n0 + 512],
                        start=False, stop=(c == EC - 1))
        for p in range(NPAIR):
            for half in range(2):
                sl = slice(half * 1024, (half + 1) * 1024)
                nc.scalar.activation(kT[p][:, sl], kps[p][:, sl], Copy)
    xkpool.release()

    # ---- prologue: v projection, transposed layout (c-outer, streams xv) ---
    xvpool = tc.alloc_tile_pool(name="xv", bufs=3)
    with tc.tile_pool(name="pro2", bufs=1, space="PSUM") as pro2:
        vps = [pro2.tile([128, S], f32, tag=f"vps{p}", name=f"vps{p}")
               for p in range(NPAIR)]
        for p in range(NPAIR):
            for n0 in range(0, S, 512):
                nc.tensor.matmul(vps[p][:, n0:n0 + 512],
                                 b_sb["bv"][0:1, p * 128:(p + 1) * 128],
                                 ones[0:1, :], start=True, stop=False)
        for c in range(EC):
            xvc = xvpool.tile([128, S], bf, tag="xv", name=f"xv{c}")
            for half in range(2):
                sl = slice(half * 1024, (half + 1) * 1024)
                nc.sync.dma_start(xvc[:, sl], inp["xv"][c][:, sl])
            for p in range(NPAIR):
                for n0 in range(0, S, 512):
                    nc.tensor.matmul(vps[p][:, n0:n0 + 512],
                                     w_sb["wv"][c][:, p * 128:(p + 1) * 128],
                                     xvc[:, n0:n0 + 512],
                                     start=False, stop=(c == EC - 1))
        for p in range(NPAIR):
            for half in range(2):
                sl = slice(half * 1024, (half + 1) * 1024)
                nc.scalar.activation(vTs[p][:, sl], vps[p][:, sl], Copy)
    xvpool.release()

    # ---- xq half-chunk pieces + mask halves in need order ------------------
    xqpool = tc.alloc_tile_pool(name="xq", bufs=8)
    xq_pc = {}
    for h in range(2):
        hsl = slice(h * 1024, (h + 1) * 1024)
        for c in range(EC):
            pc = xqpool.tile([128, 1024], bf, tag="xqp", name=f"xq{c}_{h}")
            nc.sync.dma_start(pc[:], inp["xq"][c][:, hsl])
            xq_pc[(c, h)] = pc
        if h == 0:
            for t in range(NT):
                nc.sync.dma_start(mt[t][:, 0:1024], inp["mask"][:, t, 0:1024])
    for t in range(NT):
        nc.sync.dma_start(mt[t][:, 1024:2048], inp["mask"][:, t, 1024:2048])

    # ---- attention-phase PSUM pools ---------------------------------------
    fillp = tc.alloc_tile_pool(name="fill", bufs=1, space="PSUM")
    attnp = tc.alloc_tile_pool(name="attn", bufs=1, space="PSUM")
    espool = tc.alloc_tile_pool(name="es", bufs=3)
    ysbp = tc.alloc_tile_pool(name="ysb", bufs=2)

    # --- filler units (each emits a small batch of PE work + followups) ----
    def unit_transpose(p, t, engine_act):
        def emit():
            tp = fillp.tile([128, 128], bf, tag="tp", name=f"tp{p}_{t}")
            nc.tensor.transpose(tp[:], vTs[p][:, t * 128:(t + 1) * 128],
                                ident[:])
            eng = nc.scalar if engine_act else nc.vector
            if engine_act:
                nc.scalar.activation(vA[p][t][:, 0:64], tp[:, 0:64], Copy)
                nc.scalar.activation(vA[p][t][:, 65:129], tp[:, 64:128], Copy)
            else:
                nc.vector.tensor_copy(vA[p][t][:, 0:64], tp[:, 0:64])
                nc.vector.tensor_copy(vA[p][t][:, 65:129], tp[:, 64:128])
        return emit

    def unit_qproj(p, qb, engine_act):
        def emit():
            qsl = slice(qb * QB, (qb + 1) * QB)
            qp = fillp.tile([128, QB], f32, tag="f512", name=f"qp{p}_{qb}")
            nc.tensor.matmul(qp[:], b_sb["bq"][0:1, p * 128:(p + 1) * 128],
                             ones[0:1, :], start=True, stop=False)
            for c in range(EC):
                nc.tensor.matmul(qp[:],
                                 w_sb["wq"][c][:, p * 128:(p + 1) * 128],
                                 xq_pc[(c, qb // 2)][:, (qb % 2) * QB:
                                                     (qb % 2) * QB + QB],
                                 start=False, stop=(c == EC - 1))
            if engine_act:
                nc.scalar.activation(qT[p][:, qsl], qp[:], Copy, scale=0.125)
            else:
                nc.vector.tensor_scalar_mul(qT[p][:, qsl], qp[:], 0.125)
        return emit

    def unit_yproj(s, use_act):
        def emit():
            ysb = ysbp.tile([128, E], bf, tag="ysb", name=f"ysb{s}")
            scol = slice(s * 128, (s + 1) * 128)
            for eh in range(2):
                esl = slice(eh * 512, (eh + 1) * 512)
                yp = fillp.tile([128, QB], f32, tag="f512", name=f"yp{s}_{eh}")
                for p in range(NPAIR):
                    nc.tensor.matmul(yp[:], cT[p][:, scol], wo_sb[p][:, esl],
                                     start=(p == 0), stop=(p == NPAIR - 1))
                if use_act:
                    nc.scalar.activation(ysb[:, esl], yp[:], Copy)
                else:
                    nc.vector.tensor_copy(ysb[:, esl], yp[:])
            nc.sync.dma_start(y_d[scol, 0:512], ysb[:, 0:512])
            nc.sync.dma_start(y_d[scol, 512:1024], ysb[:, 512:1024])
        return emit

    # pre-attention: q-block 0 for both pairs (ACT copies), transposes t=0..2
    for p in range(NPAIR):
        unit_qproj(p, 0, True)()
    for t in range(3):
        for p in range(NPAIR):
            unit_transpose(p, t, True)()

    # filler queue: (deadline_iter, emit)
    fillers = []
    for t in range(3, NT):
        fillers.append((t - 2, unit_transpose(0, t, False)))
    for qb in range(1, NQB):
        fillers.append((16 * qb - 8, unit_qproj(0, qb, False)))
        fillers.append((16 * qb - 7, unit_qproj(1, qb, False)))
    for t in range(NT):
        fillers.append((20 + t, unit_transpose(1, t, False)))
    for qb in range(NQB):
        base = 64 + 16 * qb + 19
        for j in range(4):
            fillers.append((base + 3 * j, unit_yproj(4 * qb + j, False)))
    fillers.sort(key=lambda x: x[0])
    fillers = list(reversed(fillers))  # pop from end

    # ---- main attention loop ----------------------------------------------
    pend_ctx = None       # closure emitting previous iteration's ctx MMs
    pend_norm2 = None     # stage-2 normalization (muls)
    cur_ctx = [None, None]

    def make_ctx(p, qb, t, es):
        def emit():
            if t == 0:
                cur_ctx[0] = attnp.tile([128, QB], f32, tag="ctxA",
                                        name=f"ctxA{p}_{qb}")
                cur_ctx[1] = attnp.tile([128, QB], f32, tag="ctxB",
                                        name=f"ctxB{p}_{qb}")
            ca, cb = cur_ctx
            nc.tensor.matmul(ca[0:65, :], vA[p][t][:, 0:65], es[:, 0:512],
                             start=(t == 0), stop=(t == NT - 1),
                             tile_position=(0, 0))
            nc.tensor.matmul(cb[0:65, :], vA[p][t][:, 65:130],
                             es[:, 512:1024],
                             start=(t == 0), stop=(t == NT - 1),
                             tile_position=(0, 0))

        return emit

    def norm_stages(p, qb, ca, cb):
        qsl = slice(qb * QB, (qb + 1) * QB)

        def s1():
            # evacuate ctx psum -> SBUF (frees banks), dens -> row 0
            nc.vector.tensor_copy(dnrow[0:1, 0:QB], ca[64:65, :])
            nc.vector.tensor_copy(dnrow[0:1, QB:2 * QB], cb[64:65, :])
            nc.vector.tensor_copy(caf[0:64, :], ca[0:64, :])
            nc.vector.tensor_copy(cbf[0:64, :], cb[0:64, :])

        def s2():
            nc.vector.reciprocal_approx_fast(rcprow[:], dnrow[:])
            nc.vector.tensor_copy(rcpbrow[:], rcprow[:])
            bc = fillp.tile([128, QB], f32, tag="f512", name=f"bc{p}_{qb}")
            nc.tensor.matmul(bc[0:64, :], ones64[:], rcpbrow[0:1, 0:QB],
                             start=True, stop=True)
            nc.tensor.matmul(bc[64:128, :], ones64[:],
                             rcpbrow[0:1, QB:2 * QB], start=True, stop=True)
            norm_bc[0] = bc

        def s3():
            bc = norm_bc[0]
            nc.vector.tensor_copy(bcs[0:64, :], bc[0:64, :])
            nc.vector.tensor_copy(bcsB[:], bc[64:128, :])
            nc.vector.tensor_mul(cT[p][0:64, qsl], caf[0:64, :],
                                 bcs[0:64, :])
            nc.vector.tensor_mul(cT[p][64:128, qsl], cbf[0:64, :],
                                 bcsB[:])
        return [s1, s2, s3]

    norm_bc = [None]

    it = 0
    normq = []
    for p in range(NPAIR if BISECT >= 2 else 0):
        for qb in range(NQB):
            qsl = slice(qb * QB, (qb + 1) * QB)
            for t in range(NT):
                tcol = slice(t * 128, (t + 1) * 128)
                st = attnp.tile([128, 1024], f32, tag="st", bufs=2,
                                name=f"st{p}_{qb}_{t}")
                nc.tensor.matmul(st[:, 0:512], kT[p][0:64, tcol],
                                 qT[p][0:64, qsl], start=True, stop=True)
                nc.tensor.matmul(st[:, 512:1024], kT[p][64:128, tcol],
                                 qT[p][64:128, qsl], start=True, stop=True)
                # one filler unit per iteration when due
                if fillers and fillers[-1][0] <= it:
                    fillers.pop()[1]()
                # flush previous iteration's ctx (software pipeline); at block
                # boundaries this is the previous block's t=15 ctx, and
                # cur_ctx still points at that block's tiles.
                if pend_ctx is not None:
                    pend_ctx()
                    pend_ctx = None
                es = espool.tile([128, 1024], bf, tag="es",
                                 name=f"es{p}_{qb}_{t}")
                nc.scalar.activation(es[:], st[:], Exp, bias=neg3[:])
                nc.vector.tensor_mul(es[:, 0:512], es[:, 0:512],
                                     mt[t][:, qsl])
                nc.vector.tensor_mul(es[:, 512:1024], es[:, 512:1024],
                                     mt[t][:, qsl])
                # one pending normalization stage per iteration, after the
                # es muls so they don't head-block the DVE queue
                if normq:
                    normq.pop(0)()
                pend_ctx = make_ctx(p, qb, t, es)
                it += 1
            # queue the previous block's normalization stages
            normq.extend(norm_stages(p, qb, cur_ctx[0], cur_ctx[1]))
    # ---- tail --------------------------------------------------------------
    if pend_ctx is not None:
        pend_ctx()
        pend_ctx = None
    while normq:
        normq.pop(0)()
    if BISECT >= 3:
        while fillers:
            fillers.pop()[1]()
    ysbp.release()
    espool.release()
    attnp.release()
    fillp.release()
    xqpool.release()
    persist.release()


def _build():
    nc = bacc.Bacc("TRN2", target_bir_lowering=False, debug=False,
                   num_devices=NCORES)
    inp = {}
    for nm in ("xq", "xk", "xv"):
        inp[nm] = nc.dram_tensor(nm, [EC, 128, S], dt.bfloat16,
                                 kind="ExternalInput").ap()
    for nm in ("wq", "wk", "wv"):
        inp[nm] = nc.dram_tensor(nm, [EC, 128, 256], dt.bfloat16,
                                 kind="ExternalInput").ap()
    for nm in ("bq", "bk", "bv"):
        inp[nm] = nc.dram_tensor(nm, [1, 256], dt.bfloat16,
                                 kind="ExternalInput").ap()
    inp["wo"] = nc.dram_tensor("wo", [NPAIR, 128, E], dt.bfloat16,
                               kind="ExternalInput").ap()
    inp["ident"] = nc.dram_tensor("ident", [128, 128], dt.bfloat16,
                                  kind="ExternalInput").ap()
    inp["mask"] = nc.dram_tensor("mask", [128, NT, S], dt.bfloat16,
                                 kind="ExternalInput").ap()
    y_d = nc.dram_tensor("y", [S, E], dt.bfloat16, kind="ExternalOutput").ap()

    with tile.TileContext(nc) as tc:
        _emit(nc, tc, inp, y_d)
    nc.compile()
    return nc


_CACHE = {}
_TRACE = False
_TRACE_CORES = (0,)
_LAST_RESULT = None


def _get_nc():
    if "nc" not in _CACHE:
        _CACHE["nc"] = _build()
    return _CACHE["nc"]


_RUNNER_CACHE = {}


def _get_runner(nc):
    """Cached jitted shard_map executable (see baseline)."""
    if id(nc) in _RUNNER_CACHE:
        return _RUNNER_CACHE[id(nc)]
    import jax
    import concourse.mybir as _mybir
    from concourse import bass2jax
    from jax.sharding import Mesh, PartitionSpec
    from jax.experimental.shard_map import shard_map

    bass2jax.install_neuronx_cc_hook()
    pid_name = nc.partition_id_tensor.name if nc.partition_id_tensor else None
    in_names, out_names, out_avals, zero_shapes = [], [], [], []
    for alloc in nc.m.functions[0].allocations:
        if not isinstance(alloc, _mybir.MemoryLocationSet):
            continue
        name = alloc.memorylocations[0].name
        if alloc.kind == "ExternalInput":
            if name != pid_name:
                in_names.append(name)
        elif alloc.kind == "ExternalOutput":
            out_names.append(name)
            shape = tuple(alloc.tensor_shape)
            dtype = _mybir.dt.np(alloc.dtype)
            out_avals.append(jax.core.ShapedArray(shape, dtype))
            zero_shapes.append((shape, dtype))
    n_params = len(in_names)
    n_outs = len(out_avals)
    all_names = in_names + out_names
    if pid_name is not None:
        all_names = all_names + [pid_name]

    def _body(*args):
        operands = list(args)
        if pid_name is not None:
            operands.append(bass2jax.partition_id_tensor())
        return tuple(bass2jax._bass_exec_p.bind(
            *operands,
            out_avals=tuple(out_avals),
            in_names=tuple(all_names),
            out_names=tuple(out_names),
            lowering_input_output_aliases=(),
            sim_require_finite=True,
            sim_require_nnan=True,
            nc=nc,
        ))

    devices = jax.devices()[:NCORES]
    mesh = Mesh(np.asarray(devices), ("core",))
    donate = tuple(range(n_params, n_params + n_outs))
    sharded = jax.jit(
        shard_map(_body, mesh=mesh,
                  in_specs=(PartitionSpec("core"),) * (n_params + n_outs),
                  out_specs=(PartitionSpec("core"),) * n_outs,
                  check_rep=False),
        donate_argnums=donate, keep_unused=True)

    def run(in_maps):
        concat_in = [np.concatenate([np.asarray(m[nm]) for m in in_maps], axis=0)
                     for nm in in_names]
        concat_zeros = [np.zeros((NCORES * s[0], *s[1:]), d)
                        for s, d in zero_shapes]
        outs = sharded(*concat_in, *concat_zeros)
        return [
            {nm: np.asarray(outs[i]).reshape(NCORES, *out_avals[i].shape)[c]
             for i, nm in enumerate(out_names)}
            for c in range(NCORES)
        ]

    _RUNNER_CACHE[id(nc)] = run
    return run


def run_sharded(query, key, value, mask, Wq, bq, Wk, bk, Wv, bv, Wo, bo):
    global _LAST_RESULT
    query, key, value = (np.asarray(a, np.float32) for a in (query, key, value))
    mask = np.asarray(mask)
    Wq, bq, Wk, bk, Wv, bv, Wo, bo = (
        np.asarray(a, np.float32) for a in (Wq, bq, Wk, bk, Wv, bv, Wo, bo))

    B = query.shape[0]
    GPB = NCORES // B                 # cores per batch
    DKL = 256                         # local head dims per core

    nc = _get_nc()

    ident = np.eye(128, dtype=BF)
    xb = {}
    for b in range(B):
        xb[b] = {
            "xq": np.ascontiguousarray(query[b].T).astype(BF).reshape(EC, 128, S),
            "xk": np.ascontiguousarray(key[b].T).astype(BF).reshape(EC, 128, S),
            "xv": np.ascontiguousarray(value[b].T).astype(BF).reshape(EC, 128, S),
            "mask": np.ascontiguousarray(
                mask[b].reshape(S, NT, 128).transpose(2, 1, 0)).astype(BF),
        }

    in_maps = []
    for c in range(NCORES):
        b, g = c // GPB, c % GPB
        sl = slice(g * DKL, (g + 1) * DKL)
        in_maps.append({
            **xb[b],
            "wq": np.ascontiguousarray(Wq[:, sl]).astype(BF).reshape(EC, 128, DKL),
            "wk": np.ascontiguousarray(Wk[:, sl]).astype(BF).reshape(EC, 128, DKL),
            "wv": np.ascontiguousarray(Wv[:, sl]).astype(BF).reshape(EC, 128, DKL),
            "bq": bq[sl].astype(BF).reshape(1, DKL),
            "bk": bk[sl].astype(BF).reshape(1, DKL),
            "bv": bv[sl].astype(BF).reshape(1, DKL),
            "wo": np.ascontiguousarray(Wo[sl, :]).astype(BF).reshape(
                NPAIR, 128, E),
            "ident": ident,
        })

    if _TRACE:
        res = bass_utils.run_bass_kernel_spmd(
            nc, in_maps, core_ids=list(range(NCORES)),
            trace=True, trace_cores=list(_TRACE_CORES))
        _LAST_RESULT = res
        results = res.results
    else:
        results = _get_runner(nc)(in_maps)

    y = np.zeros((B, S, E), np.float32)
    for c in range(NCORES):
        y[c // GPB] += results[c]["y"].astype(np.float32)
    y += bo.astype(np.float32)
    return y


def kernel(**inputs):
    return run_sharded(
        inputs["query"], inputs["key"], inputs["value"], inputs["mask"],
        inputs["Wq"], inputs["bq"], inputs["Wk"], inputs["bk"],
        inputs["Wv"], inputs["bv"], inputs["Wo"], inputs["bo"])
